# revision 1
# baseline (speedup 1.0000x reference)
"""Fused graph Fokker-Planck ODE function kernel for Trainium2 (8 NeuronCores).

Sharding: data-parallel over batch B=4 x row-halves (i in [0,256) / [256,512))
-> 8 shards.  Each core computes dh_dt for one (batch, i-half) pair.

Math (per batch; all [i,j] matrices kept transposed as [j,i] on chip so the
j-contraction matmuls need no transposes):
    S      = A * (K @ Q^T) / sqrt(D)          (elementwise mask, no -inf)
    X      = exp(S)                            (unnormalized softmax; masked
                                                scores are O(5) so no max sub)
    sg     = sigmoid(10*(E_j - E_i)) = 1 - 1/(1+exp(10*(E_j-E_i)))
    M4     = X * (1-sg)                        (M3 = X*sg is never formed:
                                                G3 = X^T@B - G4)
    G3     = M3^T @ [h | E*h | L*h | 1]       (L = log(h+1e-8))
    G4     = M4^T @ [E | L | 1]
    s_i    = r3 + r4                           (softmax denominator)
    dh[i,d] = (1/s_i) * ( G3Eh - E_i*G3h + h_i*(G4E - E_i*r4)
                        + beta_d*( G3Lh - L_i*(G3h + r4*h_i) + h_i*G4L ) )
"""

import math
import os as _os
import sys

import numpy as np

for _p in ("/opt/trn_rl_repo",):
    if _p not in sys.path:
        sys.path.insert(0, _p)

B, N, D, PED = 4, 512, 32, 16
NCORES = 8
RPC = N // 2            # i-rows per core
NJT = N // 128          # j tiles of 128
NIT = RPC // 128        # i tiles of 128
BLK = 256               # rhs block stride (padded for 1-cycle/row matmuls)
GW = 131                # used columns per G-result block
KSH = 10.0
ISD = 1.0 / math.sqrt(D)

# float32r (single-pass matmuls, pre-rounded producer tiles)
R_S = _os.environ.get("KR_S", "1") == "1"
R_ACC = _os.environ.get("KR_ACC", "1") == "1"
R_QK = _os.environ.get("KR_QK", "1") == "1"

_CACHE = {}


def _patch_act_tables():
    """Make natural_log_exp_and_others the only ACT table set containing our
    functions (exp/ln/identity/copy) so bacc emits exactly one
    ACT_TABLE_LOAD.  Dict length/order is preserved — the set INDEX is the
    runtime act_func_set_id, so entries must not be removed."""
    import concourse.bacc as bacc_mod
    if getattr(bacc_mod, "_act_tables_patched", False):
        return
    orig = bacc_mod.get_activation_tables

    def filtered(arch):
        t = orig(arch)
        target = t.get("natural_log_exp_and_others")
        if not target:
            return t
        return {k: (v if k == "natural_log_exp_and_others" else (v - target))
                for k, v in t.items()}

    bacc_mod.get_activation_tables = filtered
    bacc_mod._act_tables_patched = True


def _build_program():
    import concourse.bacc as bacc
    import concourse.tile as tile
    from concourse import mybir
    from contextlib import ExitStack

    _patch_act_tables()

    fp32 = mybir.dt.float32
    f32r = mybir.dt.float32r
    AF = mybir.ActivationFunctionType
    ADD, MUL = mybir.AluOpType.add, mybir.AluOpType.mult

    dtS = f32r if R_S else fp32
    dtA = f32r if R_ACC else fp32
    dtQ = f32r if R_QK else fp32

    nc = bacc.Bacc("TRN2", target_bir_lowering=False, debug=False,
                   num_devices=NCORES)

    def din(name, shape):
        return nc.dram_tensor(name, shape, fp32, kind="ExternalInput").ap()

    AT = din("AT", [128, NJT * RPC])     # host-permuted [p, (t i)]
    hj = din("hj", [128, NJT * D])       # host-permuted [p, (t d)]
    hi = din("hi", [128, NIT * D])       # host-permuted [p, (t d)]
    pe2 = din("pe2", [PED, N + RPC])      # [peT | peiT]
    wkq = din("wkq", [PED, 2 * D])        # [Wk | Wq]
    smalls = din("smalls", [128, 8])      # [Ej(4) | Ei(2) | bk | bq]
    rows1 = din("rows1", [1, RPC + D])    # [Ei row | beta]
    out = nc.dram_tensor("out", [128, NIT * D], fp32,
                         kind="ExternalOutput").ap()

    with tile.TileContext(nc) as tc, ExitStack() as ctx:
        cst = ctx.enter_context(tc.tile_pool(name="cst", bufs=1))
        sb = ctx.enter_context(tc.tile_pool(name="sb", bufs=1))
        keep = ctx.enter_context(tc.tile_pool(name="keep", bufs=1))
        fin = ctx.enter_context(tc.tile_pool(name="fin", bufs=1))
        pq = ctx.enter_context(tc.tile_pool(name="pq", bufs=1, space="PSUM"))
        sps = ctx.enter_context(tc.tile_pool(name="sps", bufs=1, space="PSUM"))
        fps = ctx.enter_context(tc.tile_pool(name="fps", bufs=NIT, space="PSUM"))

        # ------------- input loads (A via gpsimd queue, rest via sync) -----
        rows1_sb = cst.tile([1, RPC + D], fp32, tag="rows1_sb")
        nc.scalar.dma_start(rows1_sb[:], rows1[:])
        wkq_sb = cst.tile([PED, 2 * D], fp32, tag="wkq_sb")
        nc.scalar.dma_start(wkq_sb[:], wkq[:])
        pe_sb = cst.tile([PED, N + RPC], fp32, tag="pe_sb")
        nc.sync.dma_start(pe_sb[:, 0:N // 2], pe2[:, 0:N // 2])
        nc.scalar.dma_start(pe_sb[:, N // 2:N], pe2[:, N // 2:N])
        nc.sync.dma_start(pe_sb[:, N:N + RPC], pe2[:, N:N + RPC])
        smalls_sb = cst.tile([128, 8], fp32, tag="smalls_sb")
        nc.sync.dma_start(smalls_sb[:], smalls[:])
        hj_sb = cst.tile([128, NJT * D], fp32, tag="hj_sb")
        hv = hj_sb.rearrange("p (t d) -> p t d", d=D)
        nc.scalar.dma_start(hj_sb[:], hj[:])
        hi_all = fin.tile([128, NIT * D], fp32, tag="hi_all")
        hiv = hi_all.rearrange("p (t d) -> p t d", d=D)
        nc.scalar.dma_start(hi_all[:], hi[:])
        at_all = cst.tile([128, NJT * RPC], fp32, tag="at_all")
        HW0 = NJT * RPC // 2
        nc.sync.dma_start(at_all[:, 0:HW0], AT[:, 0:HW0])
        nc.scalar.dma_start(at_all[:, HW0:2 * HW0], AT[:, HW0:2 * HW0])

        peT_sb, peiT_sb = pe_sb[:, 0:N], pe_sb[:, N:N + RPC]
        wk_sb, wq_sb = wkq_sb[:, 0:D], wkq_sb[:, D:2 * D]
        ej_sb = smalls_sb[:, 0:NJT]
        ei_sb = smalls_sb[:, NJT:NJT + NIT]
        bk_sb = smalls_sb[0:D, 6:7]
        bq_sb = smalls_sb[0:D, 7:8]
        eirow_sb = rows1_sb[:, 0:RPC]
        betarow_sb = rows1_sb[:, RPC:RPC + D]

        zero1 = cst.tile([128, 1], fp32, tag="zero1")
        nc.vector.memset(zero1[:], 0.0)
        eps1 = cst.tile([128, 1], fp32, tag="eps1")
        nc.vector.memset(eps1[:], 1e-8)
        # dummy first ACT op: hoists the one ACT_TABLE_LOAD off the
        # critical path (it otherwise waits for the first real input)
        warm = cst.tile([128, 1], fp32, tag="warm")
        nc.scalar.activation(warm[:], zero1[:], AF.Exp, bias=zero1[:])
        if R_QK:
            wkq_r = cst.tile([PED, 2 * D], dtQ, tag="wkq_r")
            nc.vector.tensor_copy(wkq_r[:], wkq_sb[:])
            pe_r = cst.tile([PED, N + RPC], dtQ, tag="pe_r")
            nc.vector.tensor_copy(pe_r[:, 0:N], pe_sb[:, 0:N])
            nc.vector.tensor_copy(pe_r[:, N:N + RPC], pe_sb[:, N:N + RPC])
            peT_sb, peiT_sb = pe_r[:, 0:N], pe_r[:, N:N + RPC]
            wk_sb, wq_sb = wkq_r[:, 0:D], wkq_r[:, D:2 * D]
        e10_sb = cst.tile([128, NJT], fp32, tag="e10")    # 10*E_j
        nc.vector.tensor_scalar_mul(e10_sb[:], ej_sb, KSH)

        # ------------- E_i broadcast early (PE is idle here) -------------
        # f32r rank-1: single-pass, 1 cycle/row (fp32 would be 2-pass 4c/row
        # and delay Q/K behind it on the PE stream)
        ones1 = cst.tile([1, 128], dtQ, tag="ones1")
        nc.vector.memset(ones1.bitcast(fp32)[:], 1.0)
        eirow_r = cst.tile([1, RPC], dtQ, tag="eirow_r")
        nc.vector.tensor_copy(eirow_r[:], eirow_sb)
        bcps = pq.tile([128, RPC], fp32, tag="qk2")
        nc.tensor.matmul(bcps[:], ones1[:], eirow_r[:], start=True, stop=True)
        eib_sb = cst.tile([128, RPC], fp32, tag="eib")    # E_i bcast
        nc.vector.tensor_copy(eib_sb[:], bcps[:])

        # ------------- K^T, Q^T -------------
        qps = pq.tile([D, N], fp32, tag="qk")
        nc.tensor.matmul(qps[:], wq_sb, peT_sb, start=True, stop=True)
        qT_sb = cst.tile([D, N], dtS, tag="qT")
        nc.scalar.activation(qT_sb[:], qps[:], AF.Identity, bias=bq_sb,
                             scale=1.0)

        kps = pq.tile([D, RPC], fp32, tag="qk2")
        nc.tensor.matmul(kps[:], wk_sb, peiT_sb, start=True, stop=True)
        kT_sb = cst.tile([D, RPC], dtS, tag="kT")
        # (K + bk) * (1/sqrt(D))
        nc.vector.tensor_scalar(kT_sb[:], kps[:], bk_sb, ISD, op0=ADD, op1=MUL)

        # ------------- rhs blocks [h | E*h | L*h | 1 | E | L | 1 | pad] ----
        rhs_all = keep.tile([128, NJT * BLK], dtA, tag="rhs_all")
        rv = rhs_all.rearrange("p (t c) -> p t c", c=BLK)
        nc.vector.tensor_copy(rv[:, :, 0:D], hv[:])
        nc.scalar.activation(rv[:, :, 98:130], hv[:], AF.Ln, bias=eps1[:])  # L
        nc.vector.tensor_mul(rv[:, :, 2 * D:3 * D], rv[:, :, 98:130], hv[:])
        for jt in range(NJT):
            nc.vector.tensor_scalar_mul(
                rhs_all[:, jt * BLK + D:jt * BLK + 2 * D],
                hj_sb[:, jt * D:(jt + 1) * D], ej_sb[:, jt:jt + 1])  # E*h
        nc.vector.tensor_copy(rv[:, :, 97:98],
                              ej_sb.rearrange("p (t o) -> p t o", o=1))  # E
        onesjt = cst.tile([128, NJT], fp32, tag="onesjt")
        nc.vector.memset(onesjt[:], 1.0)
        ojv = onesjt.rearrange("p (t o) -> p t o", o=1)
        nc.vector.tensor_copy(rv[:, :, 96:97], ojv[:])
        nc.vector.tensor_copy(rv[:, :, 130:131], ojv[:])
        # cols 131:BLK are read by the padded matmuls but their output
        # columns are never consumed; zero them (same bit pattern in f32r)
        # so nothing reads uninitialized memory.
        nc.vector.memset(rv[:, :, GW:BLK].bitcast(fp32), 0.0)

        # ---- scores + masked exp + sign split, pipelined in two halves ----
        HW2 = 2 * RPC            # half width (2 j-tiles)
        sall = sps.tile([128, NJT * RPC], fp32, tag="sall")
        X = keep.tile([128, NJT * RPC], dtA, tag="X")
        M4 = keep.tile([128, NJT * RPC], dtA, tag="M4")
        ez = sb.tile([128, NJT * RPC], fp32, tag="ez")
        d1 = sb.tile([128, NJT * RPC], fp32, tag="d1")
        rd = sb.tile([128, NJT * RPC], fp32, tag="rd")    # 1-sg, ~18 bits
        msk = sb.tile([128, NJT * RPC], fp32, tag="msk")
        for hh in range(2):
            h0 = hh * HW2
            sl = slice(h0, h0 + HW2)
            for jt in (2 * hh, 2 * hh + 1):
                nc.tensor.matmul(sall[:, jt * RPC:(jt + 1) * RPC],
                                 qT_sb[:, jt * 128:(jt + 1) * 128],
                                 kT_sb[:], start=True, stop=True)
                nc.scalar.activation(ez[:, jt * RPC:(jt + 1) * RPC], eib_sb[:],
                                     AF.Exp, bias=e10_sb[:, jt:jt + 1],
                                     scale=-KSH)
            nc.vector.tensor_scalar_add(d1[:, sl], ez[:, sl], 1.0)
            nc.vector.reciprocal_approx_fast(rd[:, sl], d1[:, sl])
            nc.vector.tensor_mul(msk[:, sl], at_all[:, sl], sall[:, sl])
            nc.scalar.activation(X[:, sl], msk[:, sl], AF.Exp, bias=zero1[:])
            nc.vector.tensor_mul(M4[:, sl], X[:, sl], rd[:, sl])

        # ------------- beta broadcast (needed only in finals) -------------
        betarow_r = cst.tile([1, D], dtQ, tag="betarow_r")
        nc.vector.tensor_copy(betarow_r[:], betarow_sb)
        bcps2 = pq.tile([128, D], fp32, tag="qk2")
        nc.tensor.matmul(bcps2[:], ones1[:], betarow_r[:], start=True, stop=True)
        betab_sb = cst.tile([128, D], fp32, tag="betab")  # beta bcast
        nc.vector.tensor_copy(betab_sb[:], bcps2[:])

        # ------------- accumulation matmuls (lhsT in {X, M4}) -------------
        SUB = mybir.AluOpType.subtract
        g_all = fin.tile([128, NIT * GW], fp32, tag="g_all")
        for it in range(NIT):
            i0 = it * 128
            ppx = fps.tile([128, BLK], fp32, tag="ppx")
            pp4 = fps.tile([128, BLK], fp32, tag="pp4")
            for jt in range(NJT):
                st, sp = (jt == 0), (jt == NJT - 1)
                blk = rhs_all[:, jt * BLK:(jt + 1) * BLK]
                nc.tensor.matmul(ppx[:], X[:, jt * RPC + i0:jt * RPC + i0 + 128],
                                 blk, start=st, stop=sp)
                nc.tensor.matmul(pp4[:], M4[:, jt * RPC + i0:jt * RPC + i0 + 128],
                                 blk, start=st, stop=sp)
            # g block = [G3h G3Eh G3Lh r3 | G4E G4L r4];  G3 = GX - G4
            g4h = fin.tile([128, 97], fp32, tag="g4h")
            nc.vector.tensor_copy(g4h[:], pp4[:, 0:97])
            nc.vector.tensor_copy(g_all[:, it * GW + 97:(it + 1) * GW],
                                  pp4[:, 97:GW])
            nc.vector.tensor_tensor(g_all[:, it * GW:it * GW + 97],
                                    ppx[:, 0:97], g4h[:], op=SUB)

        # ------------- finals, consolidated over both i-tiles -------------
        gvw = g_all.rearrange("p (t c) -> p t c", c=GW)
        G3h, G3Eh, G3Lh = gvw[:, :, 0:D], gvw[:, :, D:2 * D], gvw[:, :, 2 * D:3 * D]
        r3, G4E = gvw[:, :, 96:97], gvw[:, :, 97:98]
        G4L, r4 = gvw[:, :, 98:130], gvw[:, :, 130:131]
        eivw = ei_sb.rearrange("p (t o) -> p t o", o=1)

        def bc(ap):  # [128, NIT, 1] -> broadcast along d
            return ap.to_broadcast((128, NIT, D))

        li_all = fin.tile([128, NIT, D], fp32, tag="li_all")
        nc.scalar.activation(li_all[:], hiv[:], AF.Ln, bias=eps1[:])

        s_all = fin.tile([128, NIT], fp32, tag="s_all")
        svw = s_all.rearrange("p (t o) -> p t o", o=1)
        nc.vector.tensor_tensor(svw[:], r3, r4, op=ADD)
        invs = fin.tile([128, NIT], fp32, tag="invs")
        nc.vector.reciprocal(invs[:], s_all[:])

        m1 = fin.tile([128, NIT], fp32, tag="m1")
        m1v = m1.rearrange("p (t o) -> p t o", o=1)
        nc.vector.tensor_tensor(m1v[:], eivw[:], r4, op=MUL)
        u_all = fin.tile([128, NIT], fp32, tag="u_all")
        uv = u_all.rearrange("p (t o) -> p t o", o=1)
        nc.vector.tensor_sub(uv[:], G4E, m1v[:])

        v1 = fin.tile([128, NIT, D], fp32, tag="v1")
        nc.vector.tensor_mul(v1[:], bc(eivw), G3h)
        t1_all = fin.tile([128, NIT, D], fp32, tag="t1_all")
        nc.vector.tensor_sub(t1_all[:], G3Eh, v1[:])
        v2 = fin.tile([128, NIT, D], fp32, tag="v2")
        nc.vector.tensor_mul(v2[:], hiv[:], bc(uv))
        t12_all = fin.tile([128, NIT, D], fp32, tag="t12_all")
        nc.vector.tensor_add(t12_all[:], t1_all[:], v2[:])

        v3 = fin.tile([128, NIT, D], fp32, tag="v3")
        nc.vector.tensor_mul(v3[:], hiv[:], bc(r4))
        w_all = fin.tile([128, NIT, D], fp32, tag="w_all")
        nc.vector.tensor_add(w_all[:], G3h, v3[:])
        z_all = fin.tile([128, NIT, D], fp32, tag="z_all")
        nc.vector.tensor_mul(z_all[:], li_all[:], w_all[:])
        e1_all = fin.tile([128, NIT, D], fp32, tag="e1_all")
        nc.vector.tensor_sub(e1_all[:], G3Lh, z_all[:])
        q_all = fin.tile([128, NIT, D], fp32, tag="q_all")
        nc.vector.tensor_mul(q_all[:], hiv[:], G4L)
        e2_all = fin.tile([128, NIT, D], fp32, tag="e2_all")
        nc.vector.tensor_add(e2_all[:], e1_all[:], q_all[:])
        bt_all = fin.tile([128, NIT, D], fp32, tag="bt_all")
        bvw = betab_sb.rearrange("p (t d) -> p t d", t=1).to_broadcast((128, NIT, D))
        nc.vector.tensor_mul(bt_all[:], e2_all[:], bvw)
        pre_all = fin.tile([128, NIT, D], fp32, tag="pre_all")
        nc.vector.tensor_add(pre_all[:], t12_all[:], bt_all[:])
        res_all = fin.tile([128, NIT, D], fp32, tag="res_all")
        iv = invs.rearrange("p (t o) -> p t o", o=1)
        nc.vector.tensor_mul(res_all[:], pre_all[:], iv.to_broadcast((128, NIT, D)))
        nc.sync.dma_start(out[:], res_all.rearrange("p t d -> p (t d)"))

    nc.compile()
    return nc


def _get_program():
    if "nc" not in _CACHE:
        _CACHE["nc"] = _build_program()
    return _CACHE["nc"]


def make_in_maps(h, pe, E, A, Wk, bk, Wq, bq, beta):
    f = lambda x: np.ascontiguousarray(np.asarray(x, dtype=np.float32))
    h, pe, E, A = f(h), f(pe), f(E), f(A)
    Wk, bk, Wq, bq, beta = f(Wk), f(bk), f(Wq), f(bq), f(beta)
    wkq = np.concatenate([Wk, Wq], axis=1)
    in_maps = []
    for c in range(NCORES):
        b, r = c // 2, c % 2
        isl = slice(r * RPC, (r + 1) * RPC)
        smalls = np.zeros((128, 8), np.float32)
        smalls[:, 0:NJT] = E.reshape(NJT, 128).T
        smalls[:, NJT:NJT + NIT] = E[isl].reshape(NIT, 128).T
        smalls[0:D, 6] = bk
        smalls[0:D, 7] = bq
        rows1 = np.concatenate([E[isl], beta]).reshape(1, RPC + D)
        atp = A[isl].T.reshape(NJT, 128, RPC).transpose(1, 0, 2)
        hjp = h[b].reshape(NJT, 128, D).transpose(1, 0, 2)
        hip = h[b, isl].reshape(NIT, 128, D).transpose(1, 0, 2)
        in_maps.append({
            "AT": f(atp.reshape(128, NJT * RPC)),
            "hj": f(hjp.reshape(128, NJT * D)),
            "hi": f(hip.reshape(128, NIT * D)),
            "pe2": f(np.concatenate([pe[b].T, pe[b, isl].T], axis=1)),
            "wkq": f(wkq),
            "smalls": smalls,
            "rows1": f(rows1),
        })
    return in_maps


def gather(results):
    out = np.empty((B, N, D), np.float32)
    for c in range(NCORES):
        b, r = c // 2, c % 2
        o = results[c]["out"].reshape(128, NIT, D).transpose(1, 0, 2)
        out[b, r * RPC:(r + 1) * RPC] = o.reshape(RPC, D)
    return out


def _axon_reset():
    try:
        import ctypes
        import jax
        lib = ctypes.CDLL("/opt/axon/libaxon_pjrt.so")
        lib.axon_reset.restype = ctypes.c_int64
        jax.devices()
        lib.axon_reset()
    except Exception:
        pass


def kernel(t=None, h=None, pe=None, E=None, A=None, Wk=None, bk=None,
           Wq=None, bq=None, beta=None, **_unused):
    from concourse.bass_utils import run_bass_kernel_spmd
    nc = _get_program()
    in_maps = make_in_maps(h, pe, E, A, Wk, bk, Wq, bq, beta)
    try:
        res = run_bass_kernel_spmd(nc, in_maps, list(range(NCORES)))
    except Exception:
        # a previously wedged NeuronCore shows up as an opaque runtime
        # error on the first execute — reset the device once and retry
        _axon_reset()
        import time as _time
        _time.sleep(2)
        res = run_bass_kernel_spmd(nc, in_maps, list(range(NCORES)))
    return gather(res.results)

